# revision 3
# baseline (speedup 1.0000x reference)
"""Trainium2 Bass kernel v2 for the char-GRU (transposed-gates design).

  y = FC(GRU_last_hidden(Embed(x)))   V=128, E=H=OUT=768, B=128, T=512.

Data-parallel over batch: 8 cores x 16 rows. Per core:

  - All weight matmuls run with the WEIGHTS as the 128x128 stationary
    operand and the state h^T as the 16-col moving operand, so gates come
    out TRANSPOSED: [gate_dim (partition), batch].  FWL makes the
    per-tile LDWEIGHTS cheap (fp8: ~27-53ns).
  - x-side: table[v,:] = emb[v] @ W_ih.T + b_ih (+ b_hh on r/z cols),
    gathered via one-hot MOVING operand against stationary table tiles,
    batched 4 steps per LDW set, accumulated directly into the gate PSUM.
  - Gate chain runs on [128, 96] tiles (128-partition elementwise),
    producing h^T directly -- no per-step transposes.
  - h^T stored bf16 (single copy, feeds both matmul + blend).
"""

import numpy as np
from contextlib import ExitStack

import concourse.bass as bass
import concourse.bacc as bacc
import concourse.tile as tile
from concourse import mybir
from concourse.bass_utils import run_bass_kernel_spmd

F32 = mybir.dt.float32
BF16 = mybir.dt.bfloat16
FP8 = mybir.dt.float8e4

V, E, H, OUT = 128, 768, 768, 768
G3 = 3 * H
B_FULL, T_FULL = 128, 512
NCORES = 8
BS = B_FULL // NCORES   # 16
KT = H // 128           # 6
NSLOT = 4               # gather batch (steps per psum group)

W_DT = FP8              # recurrent weight dtype (stationary)
H_DT = BF16             # h^T state dtype (moving)
DEBUG = False           # add step-0 intermediate dumps


def emit_kernel(ctx: ExitStack, tc: tile.TileContext, io: dict, T: int,
                REPS: int = 1):
    nc = tc.nc
    add = mybir.AluOpType.add
    mult = mybir.AluOpType.mult
    iseq = mybir.AluOpType.is_equal
    Sig = mybir.ActivationFunctionType.Sigmoid
    Tanh = mybir.ActivationFunctionType.Tanh
    Ident = mybir.ActivationFunctionType.Identity

    NSLOT = min(4, T)  # noqa: local shadow for small-T tests
    assert T % NSLOT == 0
    NG = T // NSLOT  # number of gather groups

    consts = ctx.enter_context(tc.tile_pool(name="consts", bufs=1))
    whh_sb = consts.tile([128, KT, G3], W_DT, name="whh_sb")
    table_sb = consts.tile([128, G3], BF16, name="table_sb")
    oh_sb = consts.tile([128, T * BS], BF16, name="oh_sb")
    bhnw_sb = consts.tile([1, H], BF16, name="bhnw_sb")
    ones1b = consts.tile([1, BS], BF16, name="ones1b")
    fcw_sb = consts.tile([128, KT, OUT], BF16, name="fcw_sb")
    fcb_sb = consts.tile([128, KT], F32, name="fcb_sb")
    ones128 = consts.tile([128, 128], F32, name="ones128")
    ident128 = consts.tile([128, 128], F32, name="ident128")

    state = ctx.enter_context(tc.tile_pool(name="state", bufs=1))
    hT = state.tile([128, KT, BS], H_DT, name="hT")
    hT8 = state.tile([128, KT, BS], FP8, name="hT8")

    tmp = ctx.enter_context(tc.tile_pool(name="tmp", bufs=2))
    # gather-group psum (r/z/xn classes, NSLOT steps each), ping-pong
    psg = ctx.enter_context(tc.tile_pool(name="psg", bufs=2, space="PSUM"))
    # per-step hn psum + FC tail
    psh = ctx.enter_context(tc.tile_pool(name="psh", bufs=2, space="PSUM"))

    def emit_init():
        nc.sync.dma_start(table_sb[:], io["table"][:])
        nc.sync.dma_start(bhnw_sb[:], io["bhnw"][:])
        nc.sync.dma_start(fcb_sb[:], io["fcb"][:])
        for k in range(KT):
            nc.sync.dma_start(whh_sb[:, k, :], io["whh"][k])
            nc.sync.dma_start(fcw_sb[:, k, :], io["fcw"][k])
        nq = 4
        step = (T * BS) // nq
        for q in range(nq):
            nc.sync.dma_start(oh_sb[:, q * step:(q + 1) * step],
                              io["oh"][:, q * step:(q + 1) * step])
        nc.vector.memset(ones128[:], 1.0)
        nc.vector.memset(ones1b[:], 1.0)
        nc.gpsimd.affine_select(ident128[:], ones128[:], pattern=[[1, 128]],
                                compare_op=iseq, fill=0.0, base=0,
                                channel_multiplier=-1)
        nc.vector.memset(hT[:], 0.0)
        nc.vector.memset(hT8[:], 0.0)

    # table col ranges: class c (0=r,1=z,2=n/x) tile j covers
    # gate dims c*768 + j*128 ...  +128
    def gcol(c, j):
        return c * H + j * 128

    def emit_gather_group(g, ps_r, ps_z, ps_x, tiles):
        """Gather MMs for group g (steps 4g..4g+3) for the given tile js.
        psum class layout: [128, KT, NSLOT*BS] -> gather writes [:, j, :]
        (contiguous 64), kloop writes [:, j, slot*BS:+BS] (contiguous 16)."""
        c0 = g * NSLOT * BS
        mv = oh_sb[:, c0:c0 + NSLOT * BS]
        for (c, j, dst) in tiles:
            st = table_sb[:, gcol(c, j):gcol(c, j) + 128]
            # start=True only on the bank's first write (j==0): start marks
            # the WHOLE 2KB zero-region pending; later writes then overwrite
            # on first touch and accumulate after.
            nc.tensor.matmul(dst[:, j, :], st, mv,
                             start=(j == 0), stop=False, skip_group_check=True)

    def alloc_group():
        ps_r = psg.tile([128, KT, NSLOT * BS], F32, name="ps_r", tag="gr")
        ps_z = psg.tile([128, KT, NSLOT * BS], F32, name="ps_z", tag="gz")
        ps_x = psg.tile([128, KT, NSLOT * BS], F32, name="ps_x", tag="gx")
        return ps_r, ps_z, ps_x

    def emit_body():
        emit_init()
        groups = {}
        groups[0] = alloc_group()
        emit_gather_group(0, *groups[0],
                          [(c, j, groups[0][c]) for c in range(3)
                           for j in range(KT)])

        for t in range(T):
            g, slot = t // NSLOT, t % NSLOT
            if slot == 0 and g + 1 < NG:
                groups[g + 1] = alloc_group()
            ps_r, ps_z, ps_x = groups[g]
            ps_hn = psh.tile([128, KT, BS], F32, name="ps_hn", tag="hn")

            # spread next group's gather tiles across this group's steps
            if g + 1 < NG:
                ntiles = [(c, j) for c in range(3) for j in range(KT)]
                lo = (slot * 18) // NSLOT
                hi = ((slot + 1) * 18) // NSLOT
                nxt = groups[g + 1]
                emit_gather_group(g + 1, *nxt,
                                  [(c, j, nxt[c]) for (c, j) in ntiles[lo:hi]])

            # n-gate bias rows start the hn bank (K=1 matmuls; PE filler
            # that runs during the previous step's chain wait)
            for j in range(KT):
                nc.tensor.matmul(ps_hn[:, j, :],
                                 bhnw_sb[:, j * 128:(j + 1) * 128], ones1b[:],
                                 start=(j == 0), stop=False,
                                 skip_group_check=True)
            # recurrent matmuls, kt-outer: kt 0-2 need only the A-half of
            # h^T(t-1), kt 3-5 the B-half -> chain halves pipeline.
            for kt in range(KT):
                mv = hT8[:, kt, :]
                for c in (2, 0, 1):
                    dst = (ps_r, ps_z)[c] if c < 2 else None
                    for j in range(KT):
                        st = whh_sb[:, kt, gcol(c, j):gcol(c, j) + 128]
                        if c == 2:
                            nc.tensor.matmul(ps_hn[:, j, :], st, mv,
                                             start=False,
                                             stop=(kt == KT - 1),
                                             skip_group_check=True)
                        else:
                            out = dst[:, j, slot * BS:(slot + 1) * BS]
                            nc.tensor.matmul(out, st, mv, start=False,
                                             stop=(kt == KT - 1),
                                             skip_group_check=True)

            # gate chain, split into k-halves (A: j 0-2, B: j 3-5).
            # Critical path per half: a -> b -> n -> d1 -> hT8 (5 hops);
            # r/z/u/d2 run early, bf16 hT written off-path on gpsimd.
            r_t = tmp.tile([128, KT, BS], F32, name="r_t", tag="r")
            z_t = tmp.tile([128, KT, BS], F32, name="z_t", tag="z")
            u_t = tmp.tile([128, KT, BS], F32, name="u_t", tag="u")
            a_t = tmp.tile([128, KT, BS], F32, name="a_t", tag="a")
            b_t = tmp.tile([128, KT, BS], F32, name="b_t", tag="b")
            n_t = tmp.tile([128, KT, BS], F32, name="n_t", tag="n")
            d1_t = tmp.tile([128, KT, BS], F32, name="d1_t", tag="d1")
            d2_t = tmp.tile([128, KT, BS], F32, name="d2_t", tag="d2")
            sl = slice(slot * BS, (slot + 1) * BS)

            for h2 in range(2):
                js = slice(3 * h2, 3 * h2 + 3)
                nc.scalar.activation(r_t[:, js, :], ps_r[:, js, sl], Sig)
                nc.scalar.activation(z_t[:, js, :], ps_z[:, js, sl], Sig)
                nc.gpsimd.tensor_scalar(u_t[:, js, :], z_t[:, js, :],
                                        -1.0, 1.0, mult, add)
                nc.gpsimd.tensor_tensor(d2_t[:, js, :], z_t[:, js, :],
                                        hT[:, js, :], mult)
                nc.vector.tensor_tensor(a_t[:, js, :], r_t[:, js, :],
                                        ps_hn[:, js, :], mult)
                nc.vector.tensor_tensor(b_t[:, js, :], a_t[:, js, :],
                                        ps_x[:, js, sl], add)
                nc.scalar.activation(n_t[:, js, :], b_t[:, js, :], Tanh)
                nc.vector.tensor_tensor(d1_t[:, js, :], u_t[:, js, :],
                                        n_t[:, js, :], mult)
                nc.vector.tensor_tensor(hT8[:, js, :], d1_t[:, js, :],
                                        d2_t[:, js, :], add)
                nc.gpsimd.tensor_tensor(hT[:, js, :], d1_t[:, js, :],
                                        d2_t[:, js, :], add)
            if DEBUG and t == 0:
                dbg_x = tmp.tile([128, KT, BS], F32, name="dbg_x", tag="dbx")
                nc.vector.tensor_scalar_add(dbg_x[:], ps_x[:, :, sl], 0.0)
                nc.sync.dma_start(io["dbg_x"][:], dbg_x[:])
                dbg_rp = tmp.tile([128, KT, BS], F32, name="dbg_rp", tag="dbr")
                nc.vector.tensor_scalar_add(dbg_rp[:], ps_r[:, :, sl], 0.0)
                nc.sync.dma_start(io["dbg_rp"][:], dbg_rp[:])
                nc.sync.dma_start(io["dbg_r"][:], r_t[:])
                nc.sync.dma_start(io["dbg_n"][:], n_t[:])
                nc.sync.dma_start(io["dbg_h"][:], d1_t[:])

        # ---- FC head: yT = fcW @ h + fcb, then transpose back ----
        ps_y = psh.tile([128, KT * BS], F32, name="ps_y", tag="hn")
        for ot in range(KT):
            for kt in range(KT):
                st = fcw_sb[:, kt, ot * 128:(ot + 1) * 128]
                nc.tensor.matmul(ps_y[:, ot * BS:(ot + 1) * BS], st,
                                 hT[:, kt, :],
                                 start=(ot == 0 and kt == 0),
                                 stop=(ot == KT - 1 and kt == KT - 1),
                                 skip_group_check=True)
        yT_sb = tmp.tile([128, KT * BS], F32, name="yT_sb", tag="r")
        for ot in range(KT):
            nc.scalar.activation(yT_sb[:, ot * BS:(ot + 1) * BS],
                                 ps_y[:, ot * BS:(ot + 1) * BS], Ident,
                                 bias=fcb_sb[:, ot:ot + 1])
        y_sb = tmp.tile([BS, OUT], F32, name="y_sb", tag="z")
        for ot in range(KT):
            ps_t = psh.tile([BS, 128], F32, name="ps_t", tag="hn")
            nc.tensor.transpose(ps_t[:], yT_sb[:, ot * BS:(ot + 1) * BS],
                                ident128[:])
            nc.scalar.copy(y_sb[:, ot * 128:(ot + 1) * 128], ps_t[:])
        nc.sync.dma_start(io["y"][:], y_sb[:])

    if REPS == 1:
        emit_body()
    else:
        with tc.For_i(0, REPS, 1):
            emit_body()


def build(T: int = T_FULL, num_devices: int = NCORES, reps: int = 1):
    nc = bacc.Bacc("TRN2", target_bir_lowering=False, debug=False,
                   enable_asserts=False, num_devices=num_devices)
    io = {
        "oh": nc.dram_tensor("oh", [128, T * BS], BF16,
                             kind="ExternalInput").ap(),
        "whh": nc.dram_tensor("whh", [KT, 128, G3], W_DT,
                              kind="ExternalInput").ap(),
        "table": nc.dram_tensor("table", [128, G3], BF16,
                                kind="ExternalInput").ap(),
        "bhnw": nc.dram_tensor("bhnw", [1, H], BF16,
                               kind="ExternalInput").ap(),
        "fcw": nc.dram_tensor("fcw", [KT, 128, OUT], BF16,
                              kind="ExternalInput").ap(),
        "fcb": nc.dram_tensor("fcb", [128, KT], F32,
                              kind="ExternalInput").ap(),
        "y": nc.dram_tensor("y", [BS, OUT], F32, kind="ExternalOutput").ap(),
    }
    if DEBUG:
        for nm in ("dbg_x", "dbg_rp", "dbg_r", "dbg_n", "dbg_h"):
            io[nm] = nc.dram_tensor(nm, [128, KT, BS], F32,
                                    kind="ExternalOutput").ap()
    with tile.TileContext(nc) as tc, ExitStack() as ctx:
        emit_kernel(ctx, tc, io, T, REPS=reps)
    nc.compile()
    return nc


def make_in_maps(x, emb, W_ih, W_hh, b_ih, b_hh, fc_W, fc_b,
                 T: int = T_FULL, ncores: int = NCORES):
    w_np = mybir.dt.np(W_DT)
    x = np.asarray(x).astype(np.int32)[:, :T]
    emb = np.asarray(emb, np.float32)
    W_ih = np.asarray(W_ih, np.float32)
    W_hh = np.asarray(W_hh, np.float32)
    b_ih = np.asarray(b_ih, np.float32)
    b_hh = np.asarray(b_hh, np.float32)
    fc_W = np.asarray(fc_W, np.float32)
    fc_b = np.asarray(fc_b, np.float32)

    table = emb @ W_ih.T + b_ih
    table[:, :2 * H] += b_hh[:2 * H]
    table = table.astype(mybir.dt.np(BF16))                      # [V, 3H]
    # whh[kt][p, g] = W_hh[g, kt*128+p]
    whh = np.ascontiguousarray(
        W_hh.T.reshape(KT, 128, G3)).astype(w_np)                # [KT,128,G3]
    bhnw = b_hh[2 * H:].reshape(1, H).astype(mybir.dt.np(BF16))  # [1, H]
    fcw = np.ascontiguousarray(
        fc_W.T.reshape(KT, 128, OUT)).astype(mybir.dt.np(BF16))  # [KT,128,OUT]
    fcb = np.ascontiguousarray(
        fc_b.reshape(KT, 128).T).astype(np.float32)              # [128, KT]

    shared = {"whh": whh, "table": table, "bhnw": bhnw, "fcw": fcw,
              "fcb": fcb}
    cols = np.arange(T * BS)
    in_maps = []
    for c in range(ncores):
        xs = x[c * BS:(c + 1) * BS]                          # [BS, T]
        x_tmaj = np.ascontiguousarray(xs.T).reshape(T * BS)  # t-major
        oh = np.zeros((V, T * BS), np.float32)
        oh[x_tmaj, cols] = 1.0
        in_maps.append({"oh": oh.astype(mybir.dt.np(BF16)), **shared})
    return in_maps


_CACHE = {}


def kernel(x, emb, W_ih, W_hh, b_ih, b_hh, fc_W, fc_b):
    if "nc" not in _CACHE:
        _CACHE["nc"] = build()
    nc = _CACHE["nc"]
    in_maps = make_in_maps(x, emb, W_ih, W_hh, b_ih, b_hh, fc_W, fc_b)
    res = run_bass_kernel_spmd(nc, in_maps, core_ids=list(range(NCORES)))
    y = np.concatenate([res.results[c]["y"] for c in range(NCORES)], axis=0)
    return y.astype(np.float32)
